# revision 34
# baseline (speedup 1.0000x reference)
"""Multi-head GQA attention (RoPE, causal) on 8 TRN2 NeuronCores — v3.

Problem: B=1, S=2048, DIM=2048, 32 Q heads / 8 KV heads, head_dim=64, fp32 in.

Strategy (tensor parallel over heads, no collectives):
  - Core c owns Q heads 4c..4c+3 and KV head c (GQA group == core).
  - Each core computes partial out = attn_c @ woT_c; host sums 8 partials.
  - Scores computed transposed (S^T = K_rot^T.T @ Q_rot^T) so softmax's sum
    runs over the partition axis, obtained free via a ones-column in the AV
    stationary (row 64 of AV output = sum(exp)).
  - Single interleaved pipeline; phase A(0) runs 6 accumulators in parallel
    so the PE stays dense during the input-DMA window; AV results evacuate
    to SBUF immediately so softmax normalization never blocks the next
    chunk's PSUM reuse; exp is 2-head batched on ScalarE; the wo projection
    (C) streams as soon as both head-pairs normalize a chunk.
  - PSUM plan: proj(2) + st(4) + av(2) = 8 banks; A(0) borrows st's 4.
"""
import sys

if "/opt/trn_rl_repo" not in sys.path:
    sys.path.insert(0, "/opt/trn_rl_repo")

import numpy as np

import concourse.bass as bass
import concourse.tile as tile
from concourse import bacc, mybir
from concourse.bass_utils import run_bass_kernel_spmd

# ---- problem constants (hardcoded per contract) ----
S = 2048          # sequence length
D = 2048          # model dim
NH = 32           # total Q heads
NKV = 8           # total KV heads
DH = 64           # head dim
NCORES = 8
HQ = NH // NCORES     # 4 Q heads per core
SQC = 512             # sq chunk
SKC = 128             # sk chunk
DC = 128              # d-chunk for projections
NSQ = S // SQC        # 4
NSK = S // SKC        # 16
NDC = D // DC         # 16

F32 = mybir.dt.float32
BF16 = mybir.dt.bfloat16

import os as _os
PREWARM = int(_os.environ.get("PREWARM", "18"))
NOLOAD = int(_os.environ.get("NOLOAD", "1"))

_PROGRAM_CACHE = {}


def _ldw_key(i):
    return (repr(i.ins[0]), getattr(i, "is_transpose", None),
            getattr(i, "perf_mode", None), getattr(i, "tile_position", None),
            getattr(i, "tile_size", None))


def _dedup_ldweights(nc):
    """Post-schedule peephole: drop an LDWEIGHTS whose stationary operand is
    already loaded (identical AP/mode as the previous LDWEIGHTS on the PE
    stream, immediately followed by its MATMUL). Waits/updates are spliced
    onto the following MATMUL. Pairs the scheduler separated simply keep
    their load, so this is always safe."""
    removed = 0
    for bb in nc.main_func.blocks:
        insts = bb.instructions
        last_key = None
        keep = []
        n = len(insts)
        for idx in range(n):
            i = insts[idx]
            tn = type(i).__name__
            if tn == "InstLdweights":
                key = _ldw_key(i)
                nxt = insts[idx + 1] if idx + 1 < n else None
                if (key == last_key and nxt is not None
                        and type(nxt).__name__ == "InstMatmult"
                        and repr(nxt.ins[1]) == key[0]):
                    si = i.sync_info
                    if si is not None and (si.on_wait or si.on_update):
                        nsi = nxt.sync_info
                        if nsi is None:
                            nxt.sync_info = si
                        else:
                            nsi.on_wait = list(nsi.on_wait) + list(si.on_wait)
                            nsi.on_update = (list(nsi.on_update)
                                             + list(si.on_update))
                    removed += 1
                    continue        # drop this LDWEIGHTS
                last_key = key
            elif tn == "InstMatmult":
                pass                # does not change loaded weights
            keep.append(i)
        if removed:
            bb.instructions = keep
    return removed


def _verify_weight_loads(nc):
    """Every MATMUL in final program order must be preceded (on the PE
    stream) by an LDWEIGHTS of exactly its stationary AP."""
    last = None
    for bb in nc.m.functions[0].blocks:
        for i in bb.instructions:
            tn = type(i).__name__
            if tn == "InstLdweights":
                last = repr(i.ins[0])
            elif tn == "InstMatmult":
                if repr(i.ins[1]) != last:
                    return False
    return True


def build_program():
    """Build the SPMD Bass program (identical on all 8 cores)."""
    if "nc" in _PROGRAM_CACHE:
        return _PROGRAM_CACHE["nc"]
    nc = _build_program(NOLOAD)
    if NOLOAD:
        assert _verify_weight_loads(nc), "weight-load dedup broke pairing"
    _PROGRAM_CACHE["nc"] = nc
    return nc


def _build_program(noload):
    nc = bacc.Bacc("TRN2", target_bir_lowering=False, debug=False,
                   num_devices=NCORES)

    xT = nc.dram_tensor("xT", [D, S], BF16, kind="ExternalInput")
    wq_il = nc.dram_tensor("wq_il", [128, NDC, HQ * DH], BF16,
                           kind="ExternalInput")
    wkv_il = nc.dram_tensor("wkv_il", [128, NDC, 2 * DH], BF16,
                            kind="ExternalInput")
    wo_il = nc.dram_tensor("wo_il", [128, 2, D], BF16, kind="ExternalInput")
    cos4 = nc.dram_tensor("cos4", [128, S], BF16, kind="ExternalInput")
    sin4 = nc.dram_tensor("sin4", [128, S], BF16, kind="ExternalInput")
    mask2 = nc.dram_tensor("mask2", [128, 2, SKC], BF16,
                           kind="ExternalInput")
    out = nc.dram_tensor("out", [S, D], BF16, kind="ExternalOutput")

    from concourse.masks import make_identity
    EXP = mybir.ActivationFunctionType.Exp

    def mm_noload(out_, lhsT, rhs, start, stop):
        """Emission-adjacent matmul sharing the previous one's stationary;
        the post-schedule _dedup_ldweights pass strips the redundant
        LDWEIGHTS when the scheduler kept the pair adjacent."""
        return nc.tensor.matmul(out_, lhsT, rhs, start=start, stop=stop)

    with tile.TileContext(nc) as tc:
        with tc.tile_pool(name="const", bufs=1) as cpool, \
             tc.tile_pool(name="work", bufs=2) as wpool, \
             tc.tile_pool(name="ps", bufs=2, space="PSUM") as ps:

            # ---- SBUF-resident constants / weights ----
            xfull = [cpool.tile([128, S], BF16, name=f"xfull{d}")
                     for d in range(NDC)]
            wq_t = cpool.tile([128, NDC, HQ * DH], BF16, name="wq_t")
            wkv_t = cpool.tile([128, NDC, 2 * DH], BF16, name="wkv_t")
            wo_t = cpool.tile([128, 2, D], BF16, name="wo_t")
            cos_t = cpool.tile([128, S], BF16, name="cos_t")
            sin_t = cpool.tile([128, S], BF16, name="sin_t")
            mask_t = cpool.tile([128, 2, SKC], BF16, name="mask_t")
            ident = cpool.tile([128, 128], BF16, name="ident")
            make_identity(nc, ident[:])

            # persistent intermediates
            qrot = [cpool.tile([128, S], BF16, name=f"qrot{t}") for t in range(2)]
            krot = cpool.tile([128, S], BF16, name="krot")
            vaug = cpool.tile([128, NSK, DH + 1], BF16, name="vaug")
            nc.vector.memset(vaug[:, :, DH:DH + 1], 1.0)
            attnT = [cpool.tile([128, S], BF16, name=f"attnT{t}") for t in range(2)]
            zg = cpool.tile([64, SQC], F32, name="zg")
            nc.vector.memset(zg[:], 1.0)
            zr = cpool.tile([64, SQC], F32, name="zr")
            z0 = cpool.tile([1, SQC], F32, name="z0")

            # ---- HAM prewarm: dense dummy matmuls with no DMA deps.
            # warm_w is memset-built (no gpsimd iota dependency like ident)
            # so the chain starts within ~0.5us of kernel entry.
            warm_w = cpool.tile([128, 256], BF16, name="warm_w")
            nc.vector.memset(warm_w[:], 0.25)
            scratch = ps.tile([128, 2, SQC], F32, name="warm", tag="st", bufs=2)
            nc.tensor.matmul(scratch[:, 0, 0:256], warm_w[:, 0:128],
                             warm_w[:], start=True, stop=True)
            for _ in range(PREWARM - 1):
                mm_noload(scratch[:, 0, 0:256], warm_w[:, 0:128],
                          warm_w[:], True, True)

            # ---------------- emission helpers ----------------
            def emit_dmas_pre():
                # ordered for earliest compute start; sync queue is FIFO
                nc.sync.dma_start(wkv_t[:], wkv_il.ap())
                nc.sync.dma_start(wq_t[:, 0:4, :], wq_il[:, 0:4, :])
                nc.sync.dma_start(xfull[0][:, 0:1024], xT[0:128, 0:1024])
                nc.sync.dma_start(xfull[0][:, 1024:2048], xT[0:128, 1024:2048])
                nc.sync.dma_start(xfull[1][:], xT[128:256, :])
                nc.sync.dma_start(wq_t[:, 4:8, :], wq_il[:, 4:8, :])
                nc.sync.dma_start(xfull[2][:], xT[2 * DC:3 * DC, :])
                nc.sync.dma_start(xfull[3][:], xT[3 * DC:4 * DC, :])
                nc.sync.dma_start(cos_t[:], cos4.ap())
                nc.sync.dma_start(sin_t[:], sin4.ap())
                nc.sync.dma_start(wq_t[:, 8:16, :], wq_il[:, 8:16, :])
                for d in range(4, 8):
                    nc.sync.dma_start(xfull[d][:], xT[d * DC:(d + 1) * DC, :])
                nc.sync.dma_start(mask_t[:], mask2.ap())
                for d in range(8, NDC):
                    nc.sync.dma_start(xfull[d][:], xT[d * DC:(d + 1) * DC, :])
                nc.sync.dma_start(wo_t[:], wo_il.ap())

            def rope_q(h, c0, qpair):
                """qpair: [128, 2, SQC] psum (or 2-tile list) -> qrot[h] cols
                [c0, c0+1024)."""
                qe = wpool.tile([128, 2, SQC], BF16, name="qe", tag="qe")
                if isinstance(qpair, list):
                    nc.vector.tensor_copy(qe[:, 0, :], qpair[0][:])
                    nc.vector.tensor_copy(qe[:, 1, :], qpair[1][:])
                else:
                    nc.vector.tensor_copy(qe[:], qpair[:])
                qef = qe[:].rearrange("p a b -> p (a b)")
                qsw = wpool.tile([128, 1024], BF16, name="qsw", tag="qsw")
                for g in range(4):
                    src = 32 * (g ^ 1)
                    nc.vector.tensor_copy(qsw[32 * g:32 * g + 32, :],
                                          qef[src:src + 32, :])
                nc.vector.tensor_mul(qef, qef, cos_t[:, c0:c0 + 1024])
                nc.vector.tensor_mul(qsw[:], qsw[:], sin_t[:, c0:c0 + 1024])
                nc.vector.tensor_add(qrot[h][:, c0:c0 + 1024], qef, qsw[:])

            def rope_kv(c0, kv0, kv1, jp):
                """K rope + V transpose for chunk pair at cols [c0, c0+1024)."""
                ke = wpool.tile([64, 1024], BF16, name="ke", tag="ke")
                nc.vector.tensor_copy(ke[:, 0:SQC], kv0[0:64, :])
                nc.vector.tensor_copy(ke[:, SQC:1024], kv1[0:64, :])
                vtmp = wpool.tile([64, 1024], BF16, name="vtmp", tag="vtmp")
                nc.scalar.copy(vtmp[:, 0:SQC], kv0[64:128, :])
                nc.scalar.copy(vtmp[:, SQC:1024], kv1[64:128, :])
                ksw = wpool.tile([64, 1024], BF16, name="ksw", tag="ksw")
                nc.vector.tensor_copy(ksw[0:32, :], ke[32:64, :])
                nc.vector.tensor_copy(ksw[32:64, :], ke[0:32, :])
                nc.vector.tensor_mul(ke[:], ke[:], cos_t[0:64, c0:c0 + 1024])
                nc.vector.tensor_mul(ksw[:], ksw[:], sin_t[0:64, c0:c0 + 1024])
                nc.vector.tensor_add(krot[0:64, c0:c0 + 1024], ke[:], ksw[:])
                nc.vector.tensor_copy(krot[64:128, c0:c0 + 1024],
                                      krot[0:64, c0:c0 + 1024])
                # V transpose: 8 PE transposes -> vaug chunks
                tps = [ps.tile([128, SQC], BF16, name=f"tps{j}", tag="proj")
                       for j in range(2)]
                for j in range(2):
                    for b in range(4):
                        i = 4 * (2 * jp + j) + b
                        dst = tps[j][:, 64 * b:64 * b + 64]
                        nc.tensor.transpose(dst, vtmp[:, (4 * j + b) * 128:
                                                      (4 * j + b) * 128 + 128],
                                            ident[0:64, 0:64])
                        nc.vector.tensor_copy(vaug[:, i, 0:DH], dst)

            def emit_A0():
                """jp=0: all 6 accumulators in parallel so the PE tracks the
                x DMA arrival; borrows the st tag (B hasn't started)."""
                s0, s1 = 0, SQC
                kv = [ps.tile([128, SQC], F32, name=f"kv{j}", tag="proj")
                      for j in range(2)]
                qt = [ps.tile([128, 2, SQC], F32, name=f"qtp{h}", tag="st",
                              bufs=2) for h in range(2)]
                for d in range(NDC):
                    st_, sp = (d == 0), (d == NDC - 1)
                    nc.tensor.matmul(kv[0][:], wkv_t[:, d, :],
                                     xfull[d][:, s0:s0 + SQC], start=st_, stop=sp)
                    mm_noload(kv[1][:], wkv_t[:, d, :],
                              xfull[d][:, s1:s1 + SQC], st_, sp)
                    for h in range(2):
                        w = wq_t[:, d, 128 * h:128 * h + 128]
                        nc.tensor.matmul(qt[h][:, 0, :], w,
                                         xfull[d][:, s0:s0 + SQC],
                                         start=st_, stop=sp)
                        mm_noload(qt[h][:, 1, :], w,
                                  xfull[d][:, s1:s1 + SQC], st_, sp)
                rope_kv(0, kv[0], kv[1], 0)
                for h in range(2):
                    rope_q(h, 0, qt[h])

            def emit_A1():
                """jp=1: x resident; sequential pairs on the proj tag only
                (B(0,*) owns st by now and fills PE stalls)."""
                c0 = 1024
                s0, s1 = 2 * SQC, 3 * SQC
                kv = [ps.tile([128, SQC], F32, name=f"kv{j}", tag="proj")
                      for j in range(2)]
                for d in range(NDC):
                    st_, sp = (d == 0), (d == NDC - 1)
                    nc.tensor.matmul(kv[0][:], wkv_t[:, d, :],
                                     xfull[d][:, s0:s0 + SQC], start=st_, stop=sp)
                    mm_noload(kv[1][:], wkv_t[:, d, :],
                              xfull[d][:, s1:s1 + SQC], st_, sp)
                rope_kv(c0, kv[0], kv[1], 1)
                for h in range(2):
                    qt = [ps.tile([128, SQC], F32, name=f"qt{h}{j}", tag="proj")
                          for j in range(2)]
                    for d in range(NDC):
                        st_, sp = (d == 0), (d == NDC - 1)
                        w = wq_t[:, d, 128 * h:128 * h + 128]
                        nc.tensor.matmul(qt[0][:], w, xfull[d][:, s0:s0 + SQC],
                                         start=st_, stop=sp)
                        mm_noload(qt[1][:], w, xfull[d][:, s1:s1 + SQC],
                                  st_, sp)
                    rope_q(h, c0, qt)

            def emit_B(hp, j):
                """Attention for head pair hp, sq chunk j."""
                s0 = j * SQC
                q = qrot[hp]
                av = [ps.tile([DH + 1, SQC], F32, name=f"av{h}", tag="av")
                      for h in range(2)]
                nsk_j = 4 * j + 4
                for i in range(nsk_j):
                    k0 = i * SKC
                    m = i - 4 * j
                    off = 0 if m < 1 else 128 * m
                    nw = SQC - off
                    st2 = ps.tile([128, 2, SQC], F32, name="st2", tag="st",
                                  bufs=2)
                    for h in range(2):
                        r0 = 64 * h
                        nc.tensor.matmul(st2[:, h, off:SQC],
                                         krot[r0:r0 + 64, k0:k0 + SKC],
                                         q[r0:r0 + 64, s0 + off:s0 + SQC],
                                         start=True, stop=True,
                                         tile_position=(r0, 0))
                    pt = wpool.tile([128, 2, SQC], BF16, name="pt", tag="pt",
                                    bufs=4)
                    nc.scalar.activation(pt[:, :, off:SQC], st2[:, :, off:SQC],
                                         EXP, scale=0.125)
                    if m >= 0:
                        # causal triangle is only SKC wide: cols beyond
                        # off+SKC of a diagonal block are fully unmasked
                        nc.vector.tensor_mul(pt[:, :, off:off + SKC],
                                             pt[:, :, off:off + SKC],
                                             mask_t[:])
                    nc.tensor.matmul(av[0][:, off:SQC], vaug[:, i, :],
                                     pt[:, 0, off:SQC],
                                     start=(i == 0), stop=(i == nsk_j - 1))
                    mm_noload(av[1][:, off:SQC], vaug[:, i, :],
                              pt[:, 1, off:SQC],
                              (i == 0), (i == nsk_j - 1))

                # evacuate AV to SBUF right away (frees the av banks; the
                # softmax normalize then runs off the critical PSUM path)
                avf = wpool.tile([DH + 1, 2, SQC], F32, name="avf", tag="avf")
                for h in range(2):
                    nc.vector.tensor_copy(avf[:, h, :], av[h][:])

                # softmax normalize: 1/Z broadcast, write attnT
                for h in range(2):
                    nc.vector.tensor_copy(zg[32 * h:32 * h + 1, :],
                                          avf[64:65, h, :])
                nc.vector.reciprocal_approx_fast(zr[0:33, :], zg[0:33, :])
                for h in range(2):
                    if h == 0:
                        zsrc = zr[0:1, :]
                    else:
                        nc.vector.tensor_copy(z0[:], zr[32:33, :])
                        zsrc = z0[:]
                    bc = wpool.tile([64, SQC], F32, name="bc", tag="bc")
                    nc.gpsimd.partition_broadcast(bc[:], zsrc)
                    dst = attnT[hp][64 * h:64 * h + 64, s0:s0 + SQC]
                    if h == 0:
                        # all-SBUF partitions 0:64 -> legal on gpsimd; keeps
                        # the bc -> mul chain on one queue and offloads DVE
                        nc.gpsimd.tensor_mul(dst, avf[0:64, 0, :], bc[:])
                    else:
                        nc.vector.tensor_mul(dst, avf[0:64, 1, :], bc[:])

            def emit_C(si):
                """Output projection for seq rows [128si, 128si+128)."""
                stage = wpool.tile([128, D], BF16, name="cstage", tag="cstage")
                for op in range(2):       # oi pairs (0,1) and (2,3)
                    cps = [ps.tile([128, SQC], F32, name=f"cps{k}", tag="proj")
                           for k in range(2)]
                    for t in range(2):
                        aT = attnT[t][:, si * 128:(si + 1) * 128]
                        nc.tensor.matmul(cps[0][:], aT,
                                         wo_t[:, t, (2 * op) * SQC:
                                              (2 * op + 1) * SQC],
                                         start=(t == 0), stop=(t == 1))
                        mm_noload(cps[1][:], aT,
                                  wo_t[:, t, (2 * op + 1) * SQC:
                                       (2 * op + 2) * SQC],
                                  (t == 0), (t == 1))
                    # the last chunks' evacs go on ScalarE: exp work is done
                    # by then and DVE is busy with the final normalizes
                    for k in range(2):
                        o0 = (2 * op + k) * SQC
                        if si >= 12:
                            nc.scalar.copy(stage[:, o0:o0 + SQC], cps[k][:])
                        else:
                            nc.vector.tensor_copy(stage[:, o0:o0 + SQC],
                                                  cps[k][:])
                nc.gpsimd.dma_start(out[si * 128:(si + 1) * 128, :], stage[:])

            # ---------------- the interleaved pipeline ----------------
            emit_dmas_pre()
            emit_A0()
            emit_B(0, 0)
            emit_B(0, 1)
            emit_A1()
            emit_B(1, 0)
            emit_B(1, 1)
            for si in range(0, 4):
                emit_C(si)
            emit_B(0, 2)
            emit_B(0, 3)
            for si in range(4, 8):
                emit_C(si)
            emit_B(1, 2)
            emit_B(1, 3)
            for si in range(8, 16):
                emit_C(si)

    if noload:
        removed = _dedup_ldweights(nc)
        print(f"kernel: deduped {removed} redundant LDWEIGHTS", file=sys.stderr)
    nc.compile()
    return nc


def prep_in_maps(x, freqs_cos, freqs_sin, wq, wk, wv, wo):
    """Host-side sharding / pre-transposition. Returns list of 8 in_maps."""
    import ml_dtypes
    mmd = ml_dtypes.bfloat16

    x = np.asarray(x, dtype=np.float32)
    freqs_cos = np.asarray(freqs_cos, dtype=np.float32)
    freqs_sin = np.asarray(freqs_sin, dtype=np.float32)
    wq = np.asarray(wq, dtype=np.float32)
    wk = np.asarray(wk, dtype=np.float32)
    wv = np.asarray(wv, dtype=np.float32)
    wo = np.asarray(wo, dtype=np.float32)

    xT = np.ascontiguousarray(x.reshape(S, D).T).astype(mmd)   # [D, S]

    # head-dim permutation: even lanes first, odd lanes second
    perm = np.concatenate([np.arange(0, DH, 2), np.arange(1, DH, 2)])
    wq_h = wq.reshape(NH, DH, D)[:, perm, :]               # [NH, DH, D]
    wk_h = wk.reshape(NKV, DH, D)[:, perm, :]              # [NKV, DH, D]
    wv_h = wv.reshape(NKV, DH, D)                          # not permuted

    # cos rows tiled x4; sin rows: [-sin; +sin] tiled x2 (signs baked in)
    cosT = np.ascontiguousarray(freqs_cos.T)               # [32, S]
    sinT = np.ascontiguousarray(freqs_sin.T)
    cos4 = np.ascontiguousarray(np.tile(cosT, (4, 1))).astype(mmd)
    sin4 = np.ascontiguousarray(
        np.tile(np.concatenate([-sinT, sinT], axis=0), (2, 1))).astype(mmd)

    # causal triangle (the only partially-masked SKC columns of a diagonal
    # block), duplicated for the 2 heads of a pair: mask2[p, h, f] = f >= p
    p_idx = np.arange(128)[:, None, None]
    f_idx = np.arange(SKC)[None, None, :]
    mask2 = np.broadcast_to((f_idx >= p_idx), (128, 2, SKC)).astype(mmd)
    mask2 = np.ascontiguousarray(mask2)

    in_maps = []
    for c in range(NCORES):
        wq_c = wq_h[HQ * c:HQ * (c + 1)].reshape(HQ * DH, D)   # [256, D]
        wqT_c = np.ascontiguousarray(wq_c.T).astype(mmd)       # [D, 256]
        wq_int = np.ascontiguousarray(
            wqT_c.reshape(NDC, 128, HQ * DH).transpose(1, 0, 2))
        wkv_c = np.concatenate([wk_h[c], wv_h[c]], axis=0)     # [128, D]
        wkvT_c = np.ascontiguousarray(wkv_c.T).astype(mmd)     # [D, 128]
        wkv_int = np.ascontiguousarray(
            wkvT_c.reshape(NDC, 128, 2 * DH).transpose(1, 0, 2))
        woT_c = np.ascontiguousarray(
            wo[:, HQ * DH * c:HQ * DH * (c + 1)].T).astype(mmd)  # [256, D]
        wo_int = np.ascontiguousarray(
            woT_c.reshape(2, 128, D).transpose(1, 0, 2))
        in_maps.append({
            "xT": xT, "wq_il": wq_int, "wkv_il": wkv_int, "wo_il": wo_int,
            "cos4": cos4, "sin4": sin4, "mask2": mask2,
        })
    return in_maps


def run(inputs, trace=False, trace_cores=None, tmpdir=None):
    """Compile (cached), run on 8 cores, gather. Returns (output, results)."""
    nc = build_program()
    in_maps = prep_in_maps(**inputs)
    res = run_bass_kernel_spmd(nc, in_maps, core_ids=list(range(NCORES)),
                               trace=trace, trace_cores=trace_cores,
                               tmpdir=tmpdir)
    acc = np.zeros((S, D), dtype=np.float32)
    for r in res.results:
        acc += r["out"].astype(np.float32)
    return acc.reshape(1, S, D), res


def kernel(**inputs):
    out, _ = run(inputs)
    return out


# revision 35
# speedup vs baseline: 1.3976x; 1.3976x over previous
"""Multi-head GQA attention (RoPE, causal) on 8 TRN2 NeuronCores — v3.

Problem: B=1, S=2048, DIM=2048, 32 Q heads / 8 KV heads, head_dim=64, fp32 in.

Strategy (tensor parallel over heads, no collectives):
  - Core c owns Q heads 4c..4c+3 and KV head c (GQA group == core).
  - Each core computes partial out = attn_c @ woT_c; host sums 8 partials.
  - Scores computed transposed (S^T = K_rot^T.T @ Q_rot^T) so softmax's sum
    runs over the partition axis, obtained free via a ones-column in the AV
    stationary (row 64 of AV output = sum(exp)).
  - Single interleaved pipeline; phase A(0) runs 6 accumulators in parallel
    so the PE stays dense during the input-DMA window; AV results evacuate
    to SBUF immediately so softmax normalization never blocks the next
    chunk's PSUM reuse; exp is 2-head batched on ScalarE; the wo projection
    (C) streams as soon as both head-pairs normalize a chunk.
  - PSUM plan: proj(2) + st(4) + av(2) = 8 banks; A(0) borrows st's 4.
"""
import sys

if "/opt/trn_rl_repo" not in sys.path:
    sys.path.insert(0, "/opt/trn_rl_repo")

import numpy as np

import concourse.bass as bass
import concourse.tile as tile
from concourse import bacc, mybir
from concourse.bass_utils import run_bass_kernel_spmd

# ---- problem constants (hardcoded per contract) ----
S = 2048          # sequence length
D = 2048          # model dim
NH = 32           # total Q heads
NKV = 8           # total KV heads
DH = 64           # head dim
NCORES = 8
HQ = NH // NCORES     # 4 Q heads per core
SQC = 512             # sq chunk
SKC = 128             # sk chunk
DC = 128              # d-chunk for projections
NSQ = S // SQC        # 4
NSK = S // SKC        # 16
NDC = D // DC         # 16

F32 = mybir.dt.float32
BF16 = mybir.dt.bfloat16

import os as _os
PREWARM = int(_os.environ.get("PREWARM", "18"))
NOLOAD = int(_os.environ.get("NOLOAD", "1"))

_PROGRAM_CACHE = {}


def _ldw_key(i):
    return (repr(i.ins[0]), getattr(i, "is_transpose", None),
            getattr(i, "perf_mode", None), getattr(i, "tile_position", None),
            getattr(i, "tile_size", None))


def _dedup_ldweights(nc):
    """Post-schedule peephole: drop an LDWEIGHTS whose stationary operand is
    already loaded (identical AP/mode as the previous LDWEIGHTS on the PE
    stream, immediately followed by its MATMUL). Waits/updates are spliced
    onto the following MATMUL. Pairs the scheduler separated simply keep
    their load, so this is always safe."""
    removed = 0
    for bb in nc.main_func.blocks:
        insts = bb.instructions
        last_key = None
        keep = []
        n = len(insts)
        for idx in range(n):
            i = insts[idx]
            tn = type(i).__name__
            if tn == "InstLdweights":
                key = _ldw_key(i)
                nxt = insts[idx + 1] if idx + 1 < n else None
                if (key == last_key and nxt is not None
                        and type(nxt).__name__ == "InstMatmult"
                        and repr(nxt.ins[1]) == key[0]):
                    si = i.sync_info
                    if si is not None and (si.on_wait or si.on_update):
                        nsi = nxt.sync_info
                        if nsi is None:
                            nxt.sync_info = si
                        else:
                            nsi.on_wait = list(nsi.on_wait) + list(si.on_wait)
                            nsi.on_update = (list(nsi.on_update)
                                             + list(si.on_update))
                    removed += 1
                    continue        # drop this LDWEIGHTS
                last_key = key
            elif tn == "InstMatmult":
                pass                # does not change loaded weights
            keep.append(i)
        if removed:
            bb.instructions = keep
    return removed


def _verify_weight_loads(nc):
    """Every MATMUL in final program order must be preceded (on the PE
    stream) by an LDWEIGHTS of exactly its stationary AP."""
    last = None
    for bb in nc.m.functions[0].blocks:
        for i in bb.instructions:
            tn = type(i).__name__
            if tn == "InstLdweights":
                last = repr(i.ins[0])
            elif tn == "InstMatmult":
                if repr(i.ins[1]) != last:
                    return False
    return True


def build_program():
    """Build the SPMD Bass program (identical on all 8 cores)."""
    if "nc" in _PROGRAM_CACHE:
        return _PROGRAM_CACHE["nc"]
    nc = _build_program(NOLOAD)
    if NOLOAD:
        assert _verify_weight_loads(nc), "weight-load dedup broke pairing"
    _PROGRAM_CACHE["nc"] = nc
    return nc


def _build_program(noload):
    nc = bacc.Bacc("TRN2", target_bir_lowering=False, debug=False,
                   num_devices=NCORES)

    xT = nc.dram_tensor("xT", [D, S], BF16, kind="ExternalInput")
    wq_il = nc.dram_tensor("wq_il", [128, NDC, HQ * DH], BF16,
                           kind="ExternalInput")
    wkv_il = nc.dram_tensor("wkv_il", [128, NDC, 2 * DH], BF16,
                            kind="ExternalInput")
    wo_il = nc.dram_tensor("wo_il", [128, 2, D], BF16, kind="ExternalInput")
    cos4 = nc.dram_tensor("cos4", [128, S], BF16, kind="ExternalInput")
    sin4 = nc.dram_tensor("sin4", [128, S], BF16, kind="ExternalInput")
    mask2 = nc.dram_tensor("mask2", [128, 2, SKC], BF16,
                           kind="ExternalInput")
    out = nc.dram_tensor("out", [S, D], BF16, kind="ExternalOutput")

    from concourse.masks import make_identity
    EXP = mybir.ActivationFunctionType.Exp

    def mm_noload(out_, lhsT, rhs, start, stop):
        """Emission-adjacent matmul sharing the previous one's stationary;
        the post-schedule _dedup_ldweights pass strips the redundant
        LDWEIGHTS when the scheduler kept the pair adjacent."""
        return nc.tensor.matmul(out_, lhsT, rhs, start=start, stop=stop)

    with tile.TileContext(nc) as tc:
        with tc.tile_pool(name="const", bufs=1) as cpool, \
             tc.tile_pool(name="work", bufs=2) as wpool, \
             tc.tile_pool(name="ps", bufs=2, space="PSUM") as ps:

            # ---- SBUF-resident constants / weights ----
            xfull = [cpool.tile([128, S], BF16, name=f"xfull{d}")
                     for d in range(NDC)]
            wq_t = cpool.tile([128, NDC, HQ * DH], BF16, name="wq_t")
            wkv_t = cpool.tile([128, NDC, 2 * DH], BF16, name="wkv_t")
            wo_t = cpool.tile([128, 2, D], BF16, name="wo_t")
            cos_t = cpool.tile([128, S], BF16, name="cos_t")
            sin_t = cpool.tile([128, S], BF16, name="sin_t")
            mask_t = cpool.tile([128, 2, SKC], BF16, name="mask_t")
            ident = cpool.tile([128, 128], BF16, name="ident")
            make_identity(nc, ident[:])

            # persistent intermediates
            qrot = [cpool.tile([128, S], BF16, name=f"qrot{t}") for t in range(2)]
            krot = cpool.tile([128, S], BF16, name="krot")
            vaug = cpool.tile([128, NSK, DH + 1], BF16, name="vaug")
            nc.vector.memset(vaug[:, :, DH:DH + 1], 1.0)
            attnT = [cpool.tile([128, S], BF16, name=f"attnT{t}") for t in range(2)]
            zg = cpool.tile([64, SQC], F32, name="zg")
            nc.vector.memset(zg[:], 1.0)
            zr = cpool.tile([64, SQC], F32, name="zr")
            z0 = cpool.tile([1, SQC], F32, name="z0")

            # ---- HAM prewarm: dense dummy matmuls with no DMA deps.
            # warm_w is memset-built (no gpsimd iota dependency like ident)
            # so the chain starts within ~0.5us of kernel entry.
            warm_w = cpool.tile([128, 256], BF16, name="warm_w")
            nc.vector.memset(warm_w[:], 0.25)
            scratch = ps.tile([128, 2, SQC], F32, name="warm", tag="st", bufs=2)
            nc.tensor.matmul(scratch[:, 0, 0:256], warm_w[:, 0:128],
                             warm_w[:], start=True, stop=True)
            for _ in range(PREWARM - 1):
                mm_noload(scratch[:, 0, 0:256], warm_w[:, 0:128],
                          warm_w[:], True, True)

            # ---------------- emission helpers ----------------
            def emit_dmas_pre():
                # ordered for earliest compute start; sync queue is FIFO
                nc.sync.dma_start(wkv_t[:], wkv_il.ap())
                nc.sync.dma_start(wq_t[:, 0:4, :], wq_il[:, 0:4, :])
                nc.sync.dma_start(xfull[0][:, 0:1024], xT[0:128, 0:1024])
                nc.sync.dma_start(xfull[0][:, 1024:2048], xT[0:128, 1024:2048])
                nc.sync.dma_start(xfull[1][:], xT[128:256, :])
                nc.sync.dma_start(wq_t[:, 4:8, :], wq_il[:, 4:8, :])
                nc.sync.dma_start(xfull[2][:], xT[2 * DC:3 * DC, :])
                nc.sync.dma_start(xfull[3][:], xT[3 * DC:4 * DC, :])
                nc.sync.dma_start(cos_t[:], cos4.ap())
                nc.sync.dma_start(sin_t[:], sin4.ap())
                nc.sync.dma_start(wq_t[:, 8:16, :], wq_il[:, 8:16, :])
                for d in range(4, 8):
                    nc.sync.dma_start(xfull[d][:], xT[d * DC:(d + 1) * DC, :])
                nc.sync.dma_start(mask_t[:], mask2.ap())
                for d in range(8, NDC):
                    nc.sync.dma_start(xfull[d][:], xT[d * DC:(d + 1) * DC, :])
                nc.sync.dma_start(wo_t[:], wo_il.ap())

            def rope_q(h, c0, qpair):
                """qpair: [128, 2, SQC] psum (or 2-tile list) -> qrot[h] cols
                [c0, c0+1024)."""
                qe = wpool.tile([128, 2, SQC], BF16, name="qe", tag="qe")
                if isinstance(qpair, list):
                    nc.vector.tensor_copy(qe[:, 0, :], qpair[0][:])
                    nc.vector.tensor_copy(qe[:, 1, :], qpair[1][:])
                else:
                    nc.vector.tensor_copy(qe[:], qpair[:])
                qef = qe[:].rearrange("p a b -> p (a b)")
                qsw = wpool.tile([128, 1024], BF16, name="qsw", tag="qsw")
                for g in range(4):
                    src = 32 * (g ^ 1)
                    nc.vector.tensor_copy(qsw[32 * g:32 * g + 32, :],
                                          qef[src:src + 32, :])
                nc.vector.tensor_mul(qef, qef, cos_t[:, c0:c0 + 1024])
                nc.vector.tensor_mul(qsw[:], qsw[:], sin_t[:, c0:c0 + 1024])
                nc.vector.tensor_add(qrot[h][:, c0:c0 + 1024], qef, qsw[:])

            def rope_kv(c0, kv0, kv1, jp):
                """K rope + V transpose for chunk pair at cols [c0, c0+1024)."""
                ke = wpool.tile([64, 1024], BF16, name="ke", tag="ke")
                nc.vector.tensor_copy(ke[:, 0:SQC], kv0[0:64, :])
                nc.vector.tensor_copy(ke[:, SQC:1024], kv1[0:64, :])
                vtmp = wpool.tile([64, 1024], BF16, name="vtmp", tag="vtmp")
                nc.scalar.copy(vtmp[:, 0:SQC], kv0[64:128, :])
                nc.scalar.copy(vtmp[:, SQC:1024], kv1[64:128, :])
                ksw = wpool.tile([64, 1024], BF16, name="ksw", tag="ksw")
                nc.vector.tensor_copy(ksw[0:32, :], ke[32:64, :])
                nc.vector.tensor_copy(ksw[32:64, :], ke[0:32, :])
                nc.vector.tensor_mul(ke[:], ke[:], cos_t[0:64, c0:c0 + 1024])
                nc.vector.tensor_mul(ksw[:], ksw[:], sin_t[0:64, c0:c0 + 1024])
                nc.vector.tensor_add(krot[0:64, c0:c0 + 1024], ke[:], ksw[:])
                nc.vector.tensor_copy(krot[64:128, c0:c0 + 1024],
                                      krot[0:64, c0:c0 + 1024])
                # V transpose: 8 PE transposes -> vaug chunks
                tps = [ps.tile([128, SQC], BF16, name=f"tps{j}", tag="proj")
                       for j in range(2)]
                for j in range(2):
                    for b in range(4):
                        i = 4 * (2 * jp + j) + b
                        dst = tps[j][:, 64 * b:64 * b + 64]
                        nc.tensor.transpose(dst, vtmp[:, (4 * j + b) * 128:
                                                      (4 * j + b) * 128 + 128],
                                            ident[0:64, 0:64])
                        nc.vector.tensor_copy(vaug[:, i, 0:DH], dst)

            def emit_A0():
                """jp=0: all 6 accumulators in parallel so the PE tracks the
                x DMA arrival; borrows the st tag (B hasn't started)."""
                s0, s1 = 0, SQC
                kv = [ps.tile([128, SQC], F32, name=f"kv{j}", tag="proj")
                      for j in range(2)]
                qt = [ps.tile([128, 2, SQC], F32, name=f"qtp{h}", tag="st",
                              bufs=2) for h in range(2)]
                for d in range(NDC):
                    st_, sp = (d == 0), (d == NDC - 1)
                    nc.tensor.matmul(kv[0][:], wkv_t[:, d, :],
                                     xfull[d][:, s0:s0 + SQC], start=st_, stop=sp)
                    mm_noload(kv[1][:], wkv_t[:, d, :],
                              xfull[d][:, s1:s1 + SQC], st_, sp)
                    for h in range(2):
                        w = wq_t[:, d, 128 * h:128 * h + 128]
                        nc.tensor.matmul(qt[h][:, 0, :], w,
                                         xfull[d][:, s0:s0 + SQC],
                                         start=st_, stop=sp)
                        mm_noload(qt[h][:, 1, :], w,
                                  xfull[d][:, s1:s1 + SQC], st_, sp)
                rope_kv(0, kv[0], kv[1], 0)
                for h in range(2):
                    rope_q(h, 0, qt[h])

            def emit_A1():
                """jp=1: x resident; sequential pairs on the proj tag only
                (B(0,*) owns st by now and fills PE stalls)."""
                c0 = 1024
                s0, s1 = 2 * SQC, 3 * SQC
                kv = [ps.tile([128, SQC], F32, name=f"kv{j}", tag="proj")
                      for j in range(2)]
                for d in range(NDC):
                    st_, sp = (d == 0), (d == NDC - 1)
                    nc.tensor.matmul(kv[0][:], wkv_t[:, d, :],
                                     xfull[d][:, s0:s0 + SQC], start=st_, stop=sp)
                    mm_noload(kv[1][:], wkv_t[:, d, :],
                              xfull[d][:, s1:s1 + SQC], st_, sp)
                rope_kv(c0, kv[0], kv[1], 1)
                for h in range(2):
                    qt = [ps.tile([128, SQC], F32, name=f"qt{h}{j}", tag="proj")
                          for j in range(2)]
                    for d in range(NDC):
                        st_, sp = (d == 0), (d == NDC - 1)
                        w = wq_t[:, d, 128 * h:128 * h + 128]
                        nc.tensor.matmul(qt[0][:], w, xfull[d][:, s0:s0 + SQC],
                                         start=st_, stop=sp)
                        mm_noload(qt[1][:], w, xfull[d][:, s1:s1 + SQC],
                                  st_, sp)
                    rope_q(h, c0, qt)

            def emit_B(hp, j):
                """Attention for head pair hp, sq chunk j."""
                s0 = j * SQC
                q = qrot[hp]
                av = [ps.tile([DH + 1, SQC], F32, name=f"av{h}", tag="av")
                      for h in range(2)]
                nsk_j = 4 * j + 4
                for i in range(nsk_j):
                    k0 = i * SKC
                    m = i - 4 * j
                    off = 0 if m < 1 else 128 * m
                    nw = SQC - off
                    st2 = ps.tile([128, 2, SQC], F32, name="st2", tag="st",
                                  bufs=2)
                    for h in range(2):
                        r0 = 64 * h
                        nc.tensor.matmul(st2[:, h, off:SQC],
                                         krot[r0:r0 + 64, k0:k0 + SKC],
                                         q[r0:r0 + 64, s0 + off:s0 + SQC],
                                         start=True, stop=True,
                                         tile_position=(r0, 0))
                    pt = wpool.tile([128, 2, SQC], BF16, name="pt", tag="pt",
                                    bufs=4)
                    nc.scalar.activation(pt[:, :, off:SQC], st2[:, :, off:SQC],
                                         EXP, scale=0.125)
                    if m >= 0:
                        # causal triangle is only SKC wide: cols beyond
                        # off+SKC of a diagonal block are fully unmasked
                        nc.vector.tensor_mul(pt[:, :, off:off + SKC],
                                             pt[:, :, off:off + SKC],
                                             mask_t[:])
                    nc.tensor.matmul(av[0][:, off:SQC], vaug[:, i, :],
                                     pt[:, 0, off:SQC],
                                     start=(i == 0), stop=(i == nsk_j - 1))
                    mm_noload(av[1][:, off:SQC], vaug[:, i, :],
                              pt[:, 1, off:SQC],
                              (i == 0), (i == nsk_j - 1))

                # evacuate AV to SBUF right away (frees the av banks; the
                # softmax normalize then runs off the critical PSUM path)
                avf = wpool.tile([DH + 1, 2, SQC], F32, name="avf", tag="avf")
                for h in range(2):
                    nc.vector.tensor_copy(avf[:, h, :], av[h][:])

                # softmax normalize: 1/Z broadcast, write attnT
                for h in range(2):
                    nc.vector.tensor_copy(zg[32 * h:32 * h + 1, :],
                                          avf[64:65, h, :])
                nc.vector.reciprocal_approx_fast(zr[0:33, :], zg[0:33, :])
                for h in range(2):
                    if h == 0:
                        zsrc = zr[0:1, :]
                    else:
                        nc.vector.tensor_copy(z0[:], zr[32:33, :])
                        zsrc = z0[:]
                    bc = wpool.tile([64, SQC], F32, name="bc", tag="bc")
                    nc.gpsimd.partition_broadcast(bc[:], zsrc)
                    nc.vector.tensor_mul(attnT[hp][64 * h:64 * h + 64,
                                                   s0:s0 + SQC],
                                         avf[0:64, h, :], bc[:])

            def emit_C(si):
                """Output projection for seq rows [128si, 128si+128)."""
                stage = wpool.tile([128, D], BF16, name="cstage", tag="cstage")
                for op in range(2):       # oi pairs (0,1) and (2,3)
                    cps = [ps.tile([128, SQC], F32, name=f"cps{k}", tag="proj")
                           for k in range(2)]
                    for t in range(2):
                        aT = attnT[t][:, si * 128:(si + 1) * 128]
                        nc.tensor.matmul(cps[0][:], aT,
                                         wo_t[:, t, (2 * op) * SQC:
                                              (2 * op + 1) * SQC],
                                         start=(t == 0), stop=(t == 1))
                        mm_noload(cps[1][:], aT,
                                  wo_t[:, t, (2 * op + 1) * SQC:
                                       (2 * op + 2) * SQC],
                                  (t == 0), (t == 1))
                    # the last chunks' evacs go on ScalarE: exp work is done
                    # by then and DVE is busy with the final normalizes
                    for k in range(2):
                        o0 = (2 * op + k) * SQC
                        if si >= 12:
                            nc.scalar.copy(stage[:, o0:o0 + SQC], cps[k][:])
                        else:
                            nc.vector.tensor_copy(stage[:, o0:o0 + SQC],
                                                  cps[k][:])
                nc.gpsimd.dma_start(out[si * 128:(si + 1) * 128, :], stage[:])

            # ---------------- the interleaved pipeline ----------------
            emit_dmas_pre()
            emit_A0()
            emit_B(0, 0)
            emit_B(0, 1)
            emit_A1()
            emit_B(1, 0)
            emit_B(1, 1)
            for si in range(0, 4):
                emit_C(si)
            emit_B(0, 2)
            emit_B(0, 3)
            for si in range(4, 8):
                emit_C(si)
            emit_B(1, 2)
            emit_B(1, 3)
            for si in range(8, 16):
                emit_C(si)

    if noload:
        removed = _dedup_ldweights(nc)
        print(f"kernel: deduped {removed} redundant LDWEIGHTS", file=sys.stderr)
    nc.compile()
    return nc


def prep_in_maps(x, freqs_cos, freqs_sin, wq, wk, wv, wo):
    """Host-side sharding / pre-transposition. Returns list of 8 in_maps."""
    import ml_dtypes
    mmd = ml_dtypes.bfloat16

    x = np.asarray(x, dtype=np.float32)
    freqs_cos = np.asarray(freqs_cos, dtype=np.float32)
    freqs_sin = np.asarray(freqs_sin, dtype=np.float32)
    wq = np.asarray(wq, dtype=np.float32)
    wk = np.asarray(wk, dtype=np.float32)
    wv = np.asarray(wv, dtype=np.float32)
    wo = np.asarray(wo, dtype=np.float32)

    xT = np.ascontiguousarray(x.reshape(S, D).T).astype(mmd)   # [D, S]

    # head-dim permutation: even lanes first, odd lanes second
    perm = np.concatenate([np.arange(0, DH, 2), np.arange(1, DH, 2)])
    wq_h = wq.reshape(NH, DH, D)[:, perm, :]               # [NH, DH, D]
    wk_h = wk.reshape(NKV, DH, D)[:, perm, :]              # [NKV, DH, D]
    wv_h = wv.reshape(NKV, DH, D)                          # not permuted

    # cos rows tiled x4; sin rows: [-sin; +sin] tiled x2 (signs baked in)
    cosT = np.ascontiguousarray(freqs_cos.T)               # [32, S]
    sinT = np.ascontiguousarray(freqs_sin.T)
    cos4 = np.ascontiguousarray(np.tile(cosT, (4, 1))).astype(mmd)
    sin4 = np.ascontiguousarray(
        np.tile(np.concatenate([-sinT, sinT], axis=0), (2, 1))).astype(mmd)

    # causal triangle (the only partially-masked SKC columns of a diagonal
    # block), duplicated for the 2 heads of a pair: mask2[p, h, f] = f >= p
    p_idx = np.arange(128)[:, None, None]
    f_idx = np.arange(SKC)[None, None, :]
    mask2 = np.broadcast_to((f_idx >= p_idx), (128, 2, SKC)).astype(mmd)
    mask2 = np.ascontiguousarray(mask2)

    in_maps = []
    for c in range(NCORES):
        wq_c = wq_h[HQ * c:HQ * (c + 1)].reshape(HQ * DH, D)   # [256, D]
        wqT_c = np.ascontiguousarray(wq_c.T).astype(mmd)       # [D, 256]
        wq_int = np.ascontiguousarray(
            wqT_c.reshape(NDC, 128, HQ * DH).transpose(1, 0, 2))
        wkv_c = np.concatenate([wk_h[c], wv_h[c]], axis=0)     # [128, D]
        wkvT_c = np.ascontiguousarray(wkv_c.T).astype(mmd)     # [D, 128]
        wkv_int = np.ascontiguousarray(
            wkvT_c.reshape(NDC, 128, 2 * DH).transpose(1, 0, 2))
        woT_c = np.ascontiguousarray(
            wo[:, HQ * DH * c:HQ * DH * (c + 1)].T).astype(mmd)  # [256, D]
        wo_int = np.ascontiguousarray(
            woT_c.reshape(2, 128, D).transpose(1, 0, 2))
        in_maps.append({
            "xT": xT, "wq_il": wq_int, "wkv_il": wkv_int, "wo_il": wo_int,
            "cos4": cos4, "sin4": sin4, "mask2": mask2,
        })
    return in_maps


def run(inputs, trace=False, trace_cores=None, tmpdir=None):
    """Compile (cached), run on 8 cores, gather. Returns (output, results)."""
    nc = build_program()
    in_maps = prep_in_maps(**inputs)
    res = run_bass_kernel_spmd(nc, in_maps, core_ids=list(range(NCORES)),
                               trace=trace, trace_cores=trace_cores,
                               tmpdir=tmpdir)
    acc = np.zeros((S, D), dtype=np.float32)
    for r in res.results:
        acc += r["out"].astype(np.float32)
    return acc.reshape(1, S, D), res


def kernel(**inputs):
    out, _ = run(inputs)
    return out


# revision 39
# speedup vs baseline: 1.4102x; 1.0090x over previous
"""Multi-head GQA attention (RoPE, causal) on 8 TRN2 NeuronCores — v3.

Problem: B=1, S=2048, DIM=2048, 32 Q heads / 8 KV heads, head_dim=64, fp32 in.

Strategy (tensor parallel over heads, no collectives):
  - Core c owns Q heads 4c..4c+3 and KV head c (GQA group == core).
  - Each core computes partial out = attn_c @ woT_c; host sums 8 partials.
  - Scores computed transposed (S^T = K_rot^T.T @ Q_rot^T) so softmax's sum
    runs over the partition axis, obtained free via a ones-column in the AV
    stationary (row 64 of AV output = sum(exp)).
  - Single interleaved pipeline; phase A(0) runs 6 accumulators in parallel
    so the PE stays dense during the input-DMA window; AV results evacuate
    to SBUF immediately so softmax normalization never blocks the next
    chunk's PSUM reuse; exp is 2-head batched on ScalarE; the wo projection
    (C) streams as soon as both head-pairs normalize a chunk.
  - PSUM plan: proj(2) + st(4) + av(2) = 8 banks; A(0) borrows st's 4.
"""
import sys

if "/opt/trn_rl_repo" not in sys.path:
    sys.path.insert(0, "/opt/trn_rl_repo")

import numpy as np

import concourse.bass as bass
import concourse.tile as tile
from concourse import bacc, mybir
from concourse.bass_utils import run_bass_kernel_spmd

# ---- problem constants (hardcoded per contract) ----
S = 2048          # sequence length
D = 2048          # model dim
NH = 32           # total Q heads
NKV = 8           # total KV heads
DH = 64           # head dim
NCORES = 8
HQ = NH // NCORES     # 4 Q heads per core
SQC = 512             # sq chunk
SKC = 128             # sk chunk
DC = 128              # d-chunk for projections
NSQ = S // SQC        # 4
NSK = S // SKC        # 16
NDC = D // DC         # 16

F32 = mybir.dt.float32
BF16 = mybir.dt.bfloat16

import os as _os
PREWARM = int(_os.environ.get("PREWARM", "18"))
NOLOAD = int(_os.environ.get("NOLOAD", "1"))

_PROGRAM_CACHE = {}


def _ldw_key(i):
    return (repr(i.ins[0]), getattr(i, "is_transpose", None),
            getattr(i, "perf_mode", None), getattr(i, "tile_position", None),
            getattr(i, "tile_size", None))


def _dedup_ldweights(nc):
    """Post-schedule peephole: drop an LDWEIGHTS whose stationary operand is
    already loaded (identical AP/mode as the previous LDWEIGHTS on the PE
    stream, immediately followed by its MATMUL). Waits/updates are spliced
    onto the following MATMUL. Pairs the scheduler separated simply keep
    their load, so this is always safe."""
    removed = 0
    for bb in nc.main_func.blocks:
        insts = bb.instructions
        last_key = None
        keep = []
        n = len(insts)
        for idx in range(n):
            i = insts[idx]
            tn = type(i).__name__
            if tn == "InstLdweights":
                key = _ldw_key(i)
                nxt = insts[idx + 1] if idx + 1 < n else None
                if (key == last_key and nxt is not None
                        and type(nxt).__name__ == "InstMatmult"
                        and repr(nxt.ins[1]) == key[0]):
                    si = i.sync_info
                    if si is not None and (si.on_wait or si.on_update):
                        nsi = nxt.sync_info
                        if nsi is None:
                            nxt.sync_info = si
                        else:
                            nsi.on_wait = list(nsi.on_wait) + list(si.on_wait)
                            nsi.on_update = (list(nsi.on_update)
                                             + list(si.on_update))
                    removed += 1
                    continue        # drop this LDWEIGHTS
                last_key = key
            elif tn == "InstMatmult":
                pass                # does not change loaded weights
            keep.append(i)
        if removed:
            bb.instructions = keep
    return removed


def _verify_weight_loads(nc):
    """Every MATMUL in final program order must be preceded (on the PE
    stream) by an LDWEIGHTS of exactly its stationary AP."""
    last = None
    for bb in nc.m.functions[0].blocks:
        for i in bb.instructions:
            tn = type(i).__name__
            if tn == "InstLdweights":
                last = repr(i.ins[0])
            elif tn == "InstMatmult":
                if repr(i.ins[1]) != last:
                    return False
    return True


def build_program():
    """Build the SPMD Bass program (identical on all 8 cores)."""
    if "nc" in _PROGRAM_CACHE:
        return _PROGRAM_CACHE["nc"]
    nc = _build_program(NOLOAD)
    if NOLOAD:
        assert _verify_weight_loads(nc), "weight-load dedup broke pairing"
    _PROGRAM_CACHE["nc"] = nc
    return nc


def _build_program(noload):
    nc = bacc.Bacc("TRN2", target_bir_lowering=False, debug=False,
                   num_devices=NCORES)

    xT = nc.dram_tensor("xT", [D, S], BF16, kind="ExternalInput")
    wq_il = nc.dram_tensor("wq_il", [128, NDC, HQ * DH], BF16,
                           kind="ExternalInput")
    wkv_il = nc.dram_tensor("wkv_il", [128, NDC, 2 * DH], BF16,
                            kind="ExternalInput")
    wo_il = nc.dram_tensor("wo_il", [128, 2, D], BF16, kind="ExternalInput")
    cos4 = nc.dram_tensor("cos4", [128, S], BF16, kind="ExternalInput")
    sin4 = nc.dram_tensor("sin4", [128, S], BF16, kind="ExternalInput")
    mask2 = nc.dram_tensor("mask2", [128, 2, SKC], BF16,
                           kind="ExternalInput")
    out = nc.dram_tensor("out", [S, D], BF16, kind="ExternalOutput")

    from concourse.masks import make_identity
    EXP = mybir.ActivationFunctionType.Exp

    def mm_noload(out_, lhsT, rhs, start, stop):
        """Emission-adjacent matmul sharing the previous one's stationary;
        the post-schedule _dedup_ldweights pass strips the redundant
        LDWEIGHTS when the scheduler kept the pair adjacent."""
        return nc.tensor.matmul(out_, lhsT, rhs, start=start, stop=stop)

    with tile.TileContext(nc) as tc:
        with tc.tile_pool(name="const", bufs=1) as cpool, \
             tc.tile_pool(name="work", bufs=2) as wpool, \
             tc.tile_pool(name="ps", bufs=2, space="PSUM") as ps:

            # ---- SBUF-resident constants / weights ----
            xfull = [cpool.tile([128, S], BF16, name=f"xfull{d}")
                     for d in range(NDC)]
            wq_t = cpool.tile([128, NDC, HQ * DH], BF16, name="wq_t")
            wkv_t = cpool.tile([128, NDC, 2 * DH], BF16, name="wkv_t")
            wo_t = cpool.tile([128, 2, D], BF16, name="wo_t")
            cos_t = cpool.tile([128, S], BF16, name="cos_t")
            sin_t = cpool.tile([128, S], BF16, name="sin_t")
            mask_t = cpool.tile([128, 2, SKC], BF16, name="mask_t")
            ident = cpool.tile([128, 128], BF16, name="ident")
            make_identity(nc, ident[:])

            # persistent intermediates
            qrot = [cpool.tile([128, S], BF16, name=f"qrot{t}") for t in range(2)]
            krot = cpool.tile([128, S], BF16, name="krot")
            vaug = cpool.tile([128, NSK, DH + 1], BF16, name="vaug")
            nc.vector.memset(vaug[:, :, DH:DH + 1], 1.0)
            attnT = [cpool.tile([128, S], BF16, name=f"attnT{t}") for t in range(2)]
            zg = cpool.tile([64, SQC], F32, name="zg")
            nc.vector.memset(zg[:], 1.0)
            zr = cpool.tile([64, SQC], F32, name="zr")
            z0 = cpool.tile([1, SQC], F32, name="z0")

            # ---- HAM prewarm: dense dummy matmuls with no DMA deps.
            # warm_w is memset-built (no gpsimd iota dependency like ident)
            # so the chain starts within ~0.5us of kernel entry.
            warm_w = cpool.tile([128, 256], BF16, name="warm_w")
            nc.vector.memset(warm_w[:], 0.25)
            scratch = ps.tile([128, 2, SQC], F32, name="warm", tag="st", bufs=2)
            nc.tensor.matmul(scratch[:, 0, 0:256], warm_w[:, 0:128],
                             warm_w[:], start=True, stop=True)
            for _ in range(PREWARM - 1):
                mm_noload(scratch[:, 0, 0:256], warm_w[:, 0:128],
                          warm_w[:], True, True)

            # ---------------- emission helpers ----------------
            def emit_dmas_pre():
                # ordered for earliest compute start; sync queue is FIFO
                nc.sync.dma_start(wkv_t[:], wkv_il.ap())
                nc.sync.dma_start(wq_t[:, 0:4, :], wq_il[:, 0:4, :])
                nc.sync.dma_start(xfull[0][:, 0:1024], xT[0:128, 0:1024])
                nc.sync.dma_start(xfull[0][:, 1024:2048], xT[0:128, 1024:2048])
                nc.sync.dma_start(xfull[1][:], xT[128:256, :])
                nc.sync.dma_start(wq_t[:, 4:8, :], wq_il[:, 4:8, :])
                nc.sync.dma_start(xfull[2][:], xT[2 * DC:3 * DC, :])
                nc.sync.dma_start(xfull[3][:], xT[3 * DC:4 * DC, :])
                nc.sync.dma_start(cos_t[:], cos4.ap())
                nc.sync.dma_start(sin_t[:], sin4.ap())
                nc.sync.dma_start(wq_t[:, 8:16, :], wq_il[:, 8:16, :])
                for d in range(4, 8):
                    nc.sync.dma_start(xfull[d][:], xT[d * DC:(d + 1) * DC, :])
                nc.sync.dma_start(mask_t[:], mask2.ap())
                for d in range(8, NDC):
                    nc.sync.dma_start(xfull[d][:], xT[d * DC:(d + 1) * DC, :])
                nc.sync.dma_start(wo_t[:], wo_il.ap())

            def rope_q(h, c0, qpair):
                """qpair: [128, 2, SQC] psum (or 2-tile list) -> qrot[h] cols
                [c0, c0+1024)."""
                qe = wpool.tile([128, 2, SQC], BF16, name="qe", tag="qe")
                if isinstance(qpair, list):
                    nc.vector.tensor_copy(qe[:, 0, :], qpair[0][:])
                    nc.vector.tensor_copy(qe[:, 1, :], qpair[1][:])
                else:
                    nc.vector.tensor_copy(qe[:], qpair[:])
                qef = qe[:].rearrange("p a b -> p (a b)")
                qsw = wpool.tile([128, 1024], BF16, name="qsw", tag="qsw")
                for g in range(4):
                    src = 32 * (g ^ 1)
                    nc.vector.tensor_copy(qsw[32 * g:32 * g + 32, :],
                                          qef[src:src + 32, :])
                nc.vector.tensor_mul(qef, qef, cos_t[:, c0:c0 + 1024])
                nc.vector.tensor_mul(qsw[:], qsw[:], sin_t[:, c0:c0 + 1024])
                nc.vector.tensor_add(qrot[h][:, c0:c0 + 1024], qef, qsw[:])

            def rope_kv(c0, kv0, kv1, jp):
                """K rope + V transpose for chunk pair at cols [c0, c0+1024)."""
                ke = wpool.tile([64, 1024], BF16, name="ke", tag="ke")
                nc.vector.tensor_copy(ke[:, 0:SQC], kv0[0:64, :])
                nc.vector.tensor_copy(ke[:, SQC:1024], kv1[0:64, :])
                vtmp = wpool.tile([64, 1024], BF16, name="vtmp", tag="vtmp")
                nc.scalar.copy(vtmp[:, 0:SQC], kv0[64:128, :])
                nc.scalar.copy(vtmp[:, SQC:1024], kv1[64:128, :])
                ksw = wpool.tile([64, 1024], BF16, name="ksw", tag="ksw")
                nc.vector.tensor_copy(ksw[0:32, :], ke[32:64, :])
                nc.vector.tensor_copy(ksw[32:64, :], ke[0:32, :])
                nc.vector.tensor_mul(ke[:], ke[:], cos_t[0:64, c0:c0 + 1024])
                nc.vector.tensor_mul(ksw[:], ksw[:], sin_t[0:64, c0:c0 + 1024])
                nc.vector.tensor_add(krot[0:64, c0:c0 + 1024], ke[:], ksw[:])
                nc.vector.tensor_copy(krot[64:128, c0:c0 + 1024],
                                      krot[0:64, c0:c0 + 1024])
                # V transpose: 8 PE transposes -> vaug chunks
                tps = [ps.tile([128, SQC], BF16, name=f"tps{j}", tag="proj")
                       for j in range(2)]
                for j in range(2):
                    for b in range(4):
                        i = 4 * (2 * jp + j) + b
                        dst = tps[j][:, 64 * b:64 * b + 64]
                        nc.tensor.transpose(dst, vtmp[:, (4 * j + b) * 128:
                                                      (4 * j + b) * 128 + 128],
                                            ident[0:64, 0:64])
                        nc.vector.tensor_copy(vaug[:, i, 0:DH], dst)

            def emit_A0():
                """jp=0: all 6 accumulators in parallel so the PE tracks the
                x DMA arrival; borrows the st tag (B hasn't started)."""
                s0, s1 = 0, SQC
                kv = [ps.tile([128, SQC], F32, name=f"kv{j}", tag="proj")
                      for j in range(2)]
                qt = [ps.tile([128, 2, SQC], F32, name=f"qtp{h}", tag="st",
                              bufs=2) for h in range(2)]
                for d in range(NDC):
                    st_, sp = (d == 0), (d == NDC - 1)
                    nc.tensor.matmul(kv[0][:], wkv_t[:, d, :],
                                     xfull[d][:, s0:s0 + SQC], start=st_, stop=sp)
                    mm_noload(kv[1][:], wkv_t[:, d, :],
                              xfull[d][:, s1:s1 + SQC], st_, sp)
                    for h in range(2):
                        w = wq_t[:, d, 128 * h:128 * h + 128]
                        nc.tensor.matmul(qt[h][:, 0, :], w,
                                         xfull[d][:, s0:s0 + SQC],
                                         start=st_, stop=sp)
                        mm_noload(qt[h][:, 1, :], w,
                                  xfull[d][:, s1:s1 + SQC], st_, sp)
                rope_kv(0, kv[0], kv[1], 0)
                for h in range(2):
                    rope_q(h, 0, qt[h])

            def emit_A1():
                """jp=1: x resident; sequential pairs on the proj tag only
                (B(0,*) owns st by now and fills PE stalls)."""
                c0 = 1024
                s0, s1 = 2 * SQC, 3 * SQC
                kv = [ps.tile([128, SQC], F32, name=f"kv{j}", tag="proj")
                      for j in range(2)]
                for d in range(NDC):
                    st_, sp = (d == 0), (d == NDC - 1)
                    nc.tensor.matmul(kv[0][:], wkv_t[:, d, :],
                                     xfull[d][:, s0:s0 + SQC], start=st_, stop=sp)
                    mm_noload(kv[1][:], wkv_t[:, d, :],
                              xfull[d][:, s1:s1 + SQC], st_, sp)
                rope_kv(c0, kv[0], kv[1], 1)
                for h in range(2):
                    qt = [ps.tile([128, SQC], F32, name=f"qt{h}{j}", tag="proj")
                          for j in range(2)]
                    for d in range(NDC):
                        st_, sp = (d == 0), (d == NDC - 1)
                        w = wq_t[:, d, 128 * h:128 * h + 128]
                        nc.tensor.matmul(qt[0][:], w, xfull[d][:, s0:s0 + SQC],
                                         start=st_, stop=sp)
                        mm_noload(qt[1][:], w, xfull[d][:, s1:s1 + SQC],
                                  st_, sp)
                    rope_q(h, c0, qt)

            def emit_B(hp, j):
                """Attention for head pair hp, sq chunk j."""
                s0 = j * SQC
                q = qrot[hp]
                av = [ps.tile([DH + 1, SQC], F32, name=f"av{h}", tag="av")
                      for h in range(2)]
                nsk_j = 4 * j + 4
                for i in range(nsk_j):
                    k0 = i * SKC
                    m = i - 4 * j
                    off = 0 if m < 1 else 128 * m
                    nw = SQC - off
                    st2 = ps.tile([128, 2, SQC], F32, name="st2", tag="st",
                                  bufs=2)
                    for h in range(2):
                        r0 = 64 * h
                        nc.tensor.matmul(st2[:, h, off:SQC],
                                         krot[r0:r0 + 64, k0:k0 + SKC],
                                         q[r0:r0 + 64, s0 + off:s0 + SQC],
                                         start=True, stop=True,
                                         tile_position=(r0, 0))
                    pt = wpool.tile([128, 2, SQC], BF16, name="pt", tag="pt",
                                    bufs=4)
                    nc.scalar.activation(pt[:, :, off:SQC], st2[:, :, off:SQC],
                                         EXP, scale=0.125)
                    if m >= 0:
                        # causal triangle is only SKC wide: cols beyond
                        # off+SKC of a diagonal block are fully unmasked
                        nc.vector.tensor_mul(pt[:, :, off:off + SKC],
                                             pt[:, :, off:off + SKC],
                                             mask_t[:])
                    nc.tensor.matmul(av[0][:, off:SQC], vaug[:, i, :],
                                     pt[:, 0, off:SQC],
                                     start=(i == 0), stop=(i == nsk_j - 1))
                    mm_noload(av[1][:, off:SQC], vaug[:, i, :],
                              pt[:, 1, off:SQC],
                              (i == 0), (i == nsk_j - 1))

                # evacuate AV to SBUF right away (frees the av banks; the
                # softmax normalize then runs off the critical PSUM path)
                avf = wpool.tile([DH + 1, 2, SQC], F32, name="avf", tag="avf")
                for h in range(2):
                    nc.vector.tensor_copy(avf[:, h, :], av[h][:])

                # softmax normalize: 1/Z broadcast, write attnT
                for h in range(2):
                    nc.vector.tensor_copy(zg[32 * h:32 * h + 1, :],
                                          avf[64:65, h, :])
                nc.vector.reciprocal_approx_fast(zr[0:33, :], zg[0:33, :])
                for h in range(2):
                    if h == 0:
                        zsrc = zr[0:1, :]
                    else:
                        nc.vector.tensor_copy(z0[:], zr[32:33, :])
                        zsrc = z0[:]
                    bc = wpool.tile([64, SQC], F32, name="bc", tag="bc")
                    nc.gpsimd.partition_broadcast(bc[:], zsrc)
                    nc.vector.tensor_mul(attnT[hp][64 * h:64 * h + 64,
                                                   s0:s0 + SQC],
                                         avf[0:64, h, :], bc[:])

            def emit_C(si):
                """Output projection for seq rows [128si, 128si+128)."""
                stage = wpool.tile([128, D], BF16, name="cstage", tag="cstage")
                for op in range(2):       # oi pairs (0,1) and (2,3)
                    cps = [ps.tile([128, SQC], F32, name=f"cps{k}", tag="proj")
                           for k in range(2)]
                    for t in range(2):
                        aT = attnT[t][:, si * 128:(si + 1) * 128]
                        nc.tensor.matmul(cps[0][:], aT,
                                         wo_t[:, t, (2 * op) * SQC:
                                              (2 * op + 1) * SQC],
                                         start=(t == 0), stop=(t == 1))
                        mm_noload(cps[1][:], aT,
                                  wo_t[:, t, (2 * op + 1) * SQC:
                                       (2 * op + 2) * SQC],
                                  (t == 0), (t == 1))
                    # the last chunks' evacs go on ScalarE: exp work is done
                    # by then and DVE is busy with the final normalizes
                    for k in range(2):
                        o0 = (2 * op + k) * SQC
                        if 8 <= si < 12:
                            nc.scalar.copy(stage[:, o0:o0 + SQC], cps[k][:])
                        else:
                            nc.vector.tensor_copy(stage[:, o0:o0 + SQC],
                                                  cps[k][:])
                if 8 <= si < 12:
                    # tail chunks: drain in halves so the last DMA is smaller
                    nc.gpsimd.dma_start(out[si * 128:(si + 1) * 128, 0:1024],
                                        stage[:, 0:1024])
                    nc.gpsimd.dma_start(out[si * 128:(si + 1) * 128, 1024:D],
                                        stage[:, 1024:D])
                else:
                    nc.gpsimd.dma_start(out[si * 128:(si + 1) * 128, :],
                                        stage[:])

            # ---------------- the interleaved pipeline ----------------
            emit_dmas_pre()
            emit_A0()
            emit_B(0, 0)
            emit_B(0, 1)
            emit_A1()
            emit_B(1, 0)
            emit_B(1, 1)
            for si in range(0, 4):
                emit_C(si)
            emit_B(0, 2)
            emit_B(0, 3)
            for si in range(4, 8):
                emit_C(si)
            emit_B(1, 3)
            for si in range(12, 16):
                emit_C(si)
            emit_B(1, 2)
            for si in range(8, 12):
                emit_C(si)

    if noload:
        removed = _dedup_ldweights(nc)
        print(f"kernel: deduped {removed} redundant LDWEIGHTS", file=sys.stderr)
    nc.compile()
    return nc


def prep_in_maps(x, freqs_cos, freqs_sin, wq, wk, wv, wo):
    """Host-side sharding / pre-transposition. Returns list of 8 in_maps."""
    import ml_dtypes
    mmd = ml_dtypes.bfloat16

    x = np.asarray(x, dtype=np.float32)
    freqs_cos = np.asarray(freqs_cos, dtype=np.float32)
    freqs_sin = np.asarray(freqs_sin, dtype=np.float32)
    wq = np.asarray(wq, dtype=np.float32)
    wk = np.asarray(wk, dtype=np.float32)
    wv = np.asarray(wv, dtype=np.float32)
    wo = np.asarray(wo, dtype=np.float32)

    xT = np.ascontiguousarray(x.reshape(S, D).T).astype(mmd)   # [D, S]

    # head-dim permutation: even lanes first, odd lanes second
    perm = np.concatenate([np.arange(0, DH, 2), np.arange(1, DH, 2)])
    wq_h = wq.reshape(NH, DH, D)[:, perm, :]               # [NH, DH, D]
    wk_h = wk.reshape(NKV, DH, D)[:, perm, :]              # [NKV, DH, D]
    wv_h = wv.reshape(NKV, DH, D)                          # not permuted

    # cos rows tiled x4; sin rows: [-sin; +sin] tiled x2 (signs baked in)
    cosT = np.ascontiguousarray(freqs_cos.T)               # [32, S]
    sinT = np.ascontiguousarray(freqs_sin.T)
    cos4 = np.ascontiguousarray(np.tile(cosT, (4, 1))).astype(mmd)
    sin4 = np.ascontiguousarray(
        np.tile(np.concatenate([-sinT, sinT], axis=0), (2, 1))).astype(mmd)

    # causal triangle (the only partially-masked SKC columns of a diagonal
    # block), duplicated for the 2 heads of a pair: mask2[p, h, f] = f >= p
    p_idx = np.arange(128)[:, None, None]
    f_idx = np.arange(SKC)[None, None, :]
    mask2 = np.broadcast_to((f_idx >= p_idx), (128, 2, SKC)).astype(mmd)
    mask2 = np.ascontiguousarray(mask2)

    in_maps = []
    for c in range(NCORES):
        wq_c = wq_h[HQ * c:HQ * (c + 1)].reshape(HQ * DH, D)   # [256, D]
        wqT_c = np.ascontiguousarray(wq_c.T).astype(mmd)       # [D, 256]
        wq_int = np.ascontiguousarray(
            wqT_c.reshape(NDC, 128, HQ * DH).transpose(1, 0, 2))
        wkv_c = np.concatenate([wk_h[c], wv_h[c]], axis=0)     # [128, D]
        wkvT_c = np.ascontiguousarray(wkv_c.T).astype(mmd)     # [D, 128]
        wkv_int = np.ascontiguousarray(
            wkvT_c.reshape(NDC, 128, 2 * DH).transpose(1, 0, 2))
        woT_c = np.ascontiguousarray(
            wo[:, HQ * DH * c:HQ * DH * (c + 1)].T).astype(mmd)  # [256, D]
        wo_int = np.ascontiguousarray(
            woT_c.reshape(2, 128, D).transpose(1, 0, 2))
        in_maps.append({
            "xT": xT, "wq_il": wq_int, "wkv_il": wkv_int, "wo_il": wo_int,
            "cos4": cos4, "sin4": sin4, "mask2": mask2,
        })
    return in_maps


def run(inputs, trace=False, trace_cores=None, tmpdir=None):
    """Compile (cached), run on 8 cores, gather. Returns (output, results)."""
    nc = build_program()
    in_maps = prep_in_maps(**inputs)
    res = run_bass_kernel_spmd(nc, in_maps, core_ids=list(range(NCORES)),
                               trace=trace, trace_cores=trace_cores,
                               tmpdir=tmpdir)
    acc = np.zeros((S, D), dtype=np.float32)
    for r in res.results:
        acc += r["out"].astype(np.float32)
    return acc.reshape(1, S, D), res


def kernel(**inputs):
    out, _ = run(inputs)
    return out


# revision 40
# speedup vs baseline: 1.4221x; 1.0084x over previous
"""Multi-head GQA attention (RoPE, causal) on 8 TRN2 NeuronCores — v3.

Problem: B=1, S=2048, DIM=2048, 32 Q heads / 8 KV heads, head_dim=64, fp32 in.

Strategy (tensor parallel over heads, no collectives):
  - Core c owns Q heads 4c..4c+3 and KV head c (GQA group == core).
  - Each core computes partial out = attn_c @ woT_c; host sums 8 partials.
  - Scores computed transposed (S^T = K_rot^T.T @ Q_rot^T) so softmax's sum
    runs over the partition axis, obtained free via a ones-column in the AV
    stationary (row 64 of AV output = sum(exp)).
  - Single interleaved pipeline; phase A(0) runs 6 accumulators in parallel
    so the PE stays dense during the input-DMA window; AV results evacuate
    to SBUF immediately so softmax normalization never blocks the next
    chunk's PSUM reuse; exp is 2-head batched on ScalarE; the wo projection
    (C) streams as soon as both head-pairs normalize a chunk.
  - PSUM plan: proj(2) + st(4) + av(2) = 8 banks; A(0) borrows st's 4.
"""
import sys

if "/opt/trn_rl_repo" not in sys.path:
    sys.path.insert(0, "/opt/trn_rl_repo")

import numpy as np

import concourse.bass as bass
import concourse.tile as tile
from concourse import bacc, mybir
from concourse.bass_utils import run_bass_kernel_spmd

# ---- problem constants (hardcoded per contract) ----
S = 2048          # sequence length
D = 2048          # model dim
NH = 32           # total Q heads
NKV = 8           # total KV heads
DH = 64           # head dim
NCORES = 8
HQ = NH // NCORES     # 4 Q heads per core
SQC = 512             # sq chunk
SKC = 128             # sk chunk
DC = 128              # d-chunk for projections
NSQ = S // SQC        # 4
NSK = S // SKC        # 16
NDC = D // DC         # 16

F32 = mybir.dt.float32
BF16 = mybir.dt.bfloat16

import os as _os
PREWARM = int(_os.environ.get("PREWARM", "18"))
NOLOAD = int(_os.environ.get("NOLOAD", "1"))

_PROGRAM_CACHE = {}


def _ldw_key(i):
    return (repr(i.ins[0]), getattr(i, "is_transpose", None),
            getattr(i, "perf_mode", None), getattr(i, "tile_position", None),
            getattr(i, "tile_size", None))


def _dedup_ldweights(nc):
    """Post-schedule peephole: drop an LDWEIGHTS whose stationary operand is
    already loaded (identical AP/mode as the previous LDWEIGHTS on the PE
    stream, immediately followed by its MATMUL). Waits/updates are spliced
    onto the following MATMUL. Pairs the scheduler separated simply keep
    their load, so this is always safe."""
    removed = 0
    for bb in nc.main_func.blocks:
        insts = bb.instructions
        last_key = None
        keep = []
        n = len(insts)
        for idx in range(n):
            i = insts[idx]
            tn = type(i).__name__
            if tn == "InstLdweights":
                key = _ldw_key(i)
                nxt = insts[idx + 1] if idx + 1 < n else None
                if (key == last_key and nxt is not None
                        and type(nxt).__name__ == "InstMatmult"
                        and repr(nxt.ins[1]) == key[0]):
                    si = i.sync_info
                    if si is not None and (si.on_wait or si.on_update):
                        nsi = nxt.sync_info
                        if nsi is None:
                            nxt.sync_info = si
                        else:
                            nsi.on_wait = list(nsi.on_wait) + list(si.on_wait)
                            nsi.on_update = (list(nsi.on_update)
                                             + list(si.on_update))
                    removed += 1
                    continue        # drop this LDWEIGHTS
                last_key = key
            elif tn == "InstMatmult":
                pass                # does not change loaded weights
            keep.append(i)
        if removed:
            bb.instructions = keep
    return removed


def _verify_weight_loads(nc):
    """Every MATMUL in final program order must be preceded (on the PE
    stream) by an LDWEIGHTS of exactly its stationary AP."""
    last = None
    for bb in nc.m.functions[0].blocks:
        for i in bb.instructions:
            tn = type(i).__name__
            if tn == "InstLdweights":
                last = repr(i.ins[0])
            elif tn == "InstMatmult":
                if repr(i.ins[1]) != last:
                    return False
    return True


def build_program():
    """Build the SPMD Bass program (identical on all 8 cores)."""
    if "nc" in _PROGRAM_CACHE:
        return _PROGRAM_CACHE["nc"]
    nc = _build_program(NOLOAD)
    if NOLOAD:
        assert _verify_weight_loads(nc), "weight-load dedup broke pairing"
    _PROGRAM_CACHE["nc"] = nc
    return nc


def _build_program(noload):
    nc = bacc.Bacc("TRN2", target_bir_lowering=False, debug=False,
                   num_devices=NCORES)

    xT = nc.dram_tensor("xT", [D, S], BF16, kind="ExternalInput")
    wq_il = nc.dram_tensor("wq_il", [128, NDC, HQ * DH], BF16,
                           kind="ExternalInput")
    wkv_il = nc.dram_tensor("wkv_il", [128, NDC, 2 * DH], BF16,
                            kind="ExternalInput")
    wo_il = nc.dram_tensor("wo_il", [128, 2, D], BF16, kind="ExternalInput")
    cos4 = nc.dram_tensor("cos4", [128, S], BF16, kind="ExternalInput")
    sin4 = nc.dram_tensor("sin4", [128, S], BF16, kind="ExternalInput")
    mask2 = nc.dram_tensor("mask2", [128, 2, SKC], BF16,
                           kind="ExternalInput")
    out = nc.dram_tensor("out", [S, D], BF16, kind="ExternalOutput")

    from concourse.masks import make_identity
    EXP = mybir.ActivationFunctionType.Exp

    def mm_noload(out_, lhsT, rhs, start, stop):
        """Emission-adjacent matmul sharing the previous one's stationary;
        the post-schedule _dedup_ldweights pass strips the redundant
        LDWEIGHTS when the scheduler kept the pair adjacent."""
        return nc.tensor.matmul(out_, lhsT, rhs, start=start, stop=stop)

    with tile.TileContext(nc) as tc:
        with tc.tile_pool(name="const", bufs=1) as cpool, \
             tc.tile_pool(name="work", bufs=2) as wpool, \
             tc.tile_pool(name="ps", bufs=2, space="PSUM") as ps:

            # ---- SBUF-resident constants / weights ----
            xfull = [cpool.tile([128, S], BF16, name=f"xfull{d}")
                     for d in range(NDC)]
            wq_t = cpool.tile([128, NDC, HQ * DH], BF16, name="wq_t")
            wkv_t = cpool.tile([128, NDC, 2 * DH], BF16, name="wkv_t")
            wo_t = cpool.tile([128, 2, D], BF16, name="wo_t")
            cos_t = cpool.tile([128, S], BF16, name="cos_t")
            sin_t = cpool.tile([128, S], BF16, name="sin_t")
            mask_t = cpool.tile([128, 2, SKC], BF16, name="mask_t")
            ident = cpool.tile([128, 128], BF16, name="ident")
            make_identity(nc, ident[:])

            # persistent intermediates
            qrot = [cpool.tile([128, S], BF16, name=f"qrot{t}") for t in range(2)]
            krot = cpool.tile([128, S], BF16, name="krot")
            vaug = cpool.tile([128, NSK, DH + 1], BF16, name="vaug")
            nc.vector.memset(vaug[:, :, DH:DH + 1], 1.0)
            attnT = [cpool.tile([128, S], BF16, name=f"attnT{t}") for t in range(2)]
            zg = cpool.tile([64, SQC], F32, name="zg")
            nc.vector.memset(zg[:], 1.0)
            zr = cpool.tile([64, SQC], F32, name="zr")
            z0 = cpool.tile([1, SQC], F32, name="z0")

            # ---- HAM prewarm: dense dummy matmuls with no DMA deps.
            # warm_w is memset-built (no gpsimd iota dependency like ident)
            # so the chain starts within ~0.5us of kernel entry.
            warm_w = cpool.tile([128, 256], BF16, name="warm_w")
            nc.vector.memset(warm_w[:], 0.25)
            scratch = ps.tile([128, 2, SQC], F32, name="warm", tag="st", bufs=2)
            nc.tensor.matmul(scratch[:, 0, 0:256], warm_w[:, 0:128],
                             warm_w[:], start=True, stop=True)
            for _ in range(PREWARM - 1):
                mm_noload(scratch[:, 0, 0:256], warm_w[:, 0:128],
                          warm_w[:], True, True)

            # ---------------- emission helpers ----------------
            def emit_dmas_pre():
                # ordered for earliest compute start; sync queue is FIFO
                nc.sync.dma_start(wkv_t[:], wkv_il.ap())
                nc.sync.dma_start(wq_t[:, 0:4, :], wq_il[:, 0:4, :])
                nc.sync.dma_start(xfull[0][:, 0:1024], xT[0:128, 0:1024])
                nc.sync.dma_start(xfull[0][:, 1024:2048], xT[0:128, 1024:2048])
                nc.sync.dma_start(xfull[1][:], xT[128:256, :])
                nc.sync.dma_start(wq_t[:, 4:8, :], wq_il[:, 4:8, :])
                nc.sync.dma_start(xfull[2][:], xT[2 * DC:3 * DC, :])
                nc.sync.dma_start(xfull[3][:], xT[3 * DC:4 * DC, :])
                nc.sync.dma_start(cos_t[:], cos4.ap())
                nc.sync.dma_start(sin_t[:], sin4.ap())
                nc.sync.dma_start(wq_t[:, 8:16, :], wq_il[:, 8:16, :])
                for d in range(4, 8):
                    nc.sync.dma_start(xfull[d][:], xT[d * DC:(d + 1) * DC, :])
                nc.sync.dma_start(mask_t[:], mask2.ap())
                for d in range(8, NDC):
                    nc.sync.dma_start(xfull[d][:], xT[d * DC:(d + 1) * DC, :])
                nc.sync.dma_start(wo_t[:], wo_il.ap())

            def rope_q(h, c0, qpair):
                """qpair: [128, 2, SQC] psum (or 2-tile list) -> qrot[h] cols
                [c0, c0+1024)."""
                qe = wpool.tile([128, 2, SQC], BF16, name="qe", tag="qe")
                if isinstance(qpair, list):
                    nc.vector.tensor_copy(qe[:, 0, :], qpair[0][:])
                    nc.vector.tensor_copy(qe[:, 1, :], qpair[1][:])
                else:
                    nc.vector.tensor_copy(qe[:], qpair[:])
                qef = qe[:].rearrange("p a b -> p (a b)")
                qsw = wpool.tile([128, 1024], BF16, name="qsw", tag="qsw")
                for g in range(4):
                    src = 32 * (g ^ 1)
                    nc.vector.tensor_copy(qsw[32 * g:32 * g + 32, :],
                                          qef[src:src + 32, :])
                nc.vector.tensor_mul(qef, qef, cos_t[:, c0:c0 + 1024])
                nc.vector.tensor_mul(qsw[:], qsw[:], sin_t[:, c0:c0 + 1024])
                nc.vector.tensor_add(qrot[h][:, c0:c0 + 1024], qef, qsw[:])

            def rope_kv(c0, kv0, kv1, jp):
                """K rope + V transpose for chunk pair at cols [c0, c0+1024)."""
                ke = wpool.tile([64, 1024], BF16, name="ke", tag="ke")
                nc.vector.tensor_copy(ke[:, 0:SQC], kv0[0:64, :])
                nc.vector.tensor_copy(ke[:, SQC:1024], kv1[0:64, :])
                vtmp = wpool.tile([64, 1024], BF16, name="vtmp", tag="vtmp")
                nc.scalar.copy(vtmp[:, 0:SQC], kv0[64:128, :])
                nc.scalar.copy(vtmp[:, SQC:1024], kv1[64:128, :])
                ksw = wpool.tile([64, 1024], BF16, name="ksw", tag="ksw")
                nc.vector.tensor_copy(ksw[0:32, :], ke[32:64, :])
                nc.vector.tensor_copy(ksw[32:64, :], ke[0:32, :])
                nc.vector.tensor_mul(ke[:], ke[:], cos_t[0:64, c0:c0 + 1024])
                nc.vector.tensor_mul(ksw[:], ksw[:], sin_t[0:64, c0:c0 + 1024])
                nc.vector.tensor_add(krot[0:64, c0:c0 + 1024], ke[:], ksw[:])
                nc.vector.tensor_copy(krot[64:128, c0:c0 + 1024],
                                      krot[0:64, c0:c0 + 1024])
                # V transpose: 8 PE transposes -> vaug chunks
                tps = [ps.tile([128, SQC], BF16, name=f"tps{j}", tag="proj")
                       for j in range(2)]
                for j in range(2):
                    for b in range(4):
                        i = 4 * (2 * jp + j) + b
                        dst = tps[j][:, 64 * b:64 * b + 64]
                        nc.tensor.transpose(dst, vtmp[:, (4 * j + b) * 128:
                                                      (4 * j + b) * 128 + 128],
                                            ident[0:64, 0:64])
                        nc.vector.tensor_copy(vaug[:, i, 0:DH], dst)

            def emit_A0():
                """jp=0: all 6 accumulators in parallel so the PE tracks the
                x DMA arrival; borrows the st tag (B hasn't started)."""
                s0, s1 = 0, SQC
                kv = [ps.tile([128, SQC], F32, name=f"kv{j}", tag="proj")
                      for j in range(2)]
                qt = [ps.tile([128, 2, SQC], F32, name=f"qtp{h}", tag="st",
                              bufs=2) for h in range(2)]
                for d in range(NDC):
                    st_, sp = (d == 0), (d == NDC - 1)
                    nc.tensor.matmul(kv[0][:], wkv_t[:, d, :],
                                     xfull[d][:, s0:s0 + SQC], start=st_, stop=sp)
                    mm_noload(kv[1][:], wkv_t[:, d, :],
                              xfull[d][:, s1:s1 + SQC], st_, sp)
                    for h in range(2):
                        w = wq_t[:, d, 128 * h:128 * h + 128]
                        nc.tensor.matmul(qt[h][:, 0, :], w,
                                         xfull[d][:, s0:s0 + SQC],
                                         start=st_, stop=sp)
                        mm_noload(qt[h][:, 1, :], w,
                                  xfull[d][:, s1:s1 + SQC], st_, sp)
                rope_kv(0, kv[0], kv[1], 0)
                for h in range(2):
                    rope_q(h, 0, qt[h])

            def emit_A1():
                """jp=1: x resident; sequential pairs on the proj tag only
                (B(0,*) owns st by now and fills PE stalls)."""
                c0 = 1024
                s0, s1 = 2 * SQC, 3 * SQC
                kv = [ps.tile([128, SQC], F32, name=f"kv{j}", tag="proj")
                      for j in range(2)]
                for d in range(NDC):
                    st_, sp = (d == 0), (d == NDC - 1)
                    nc.tensor.matmul(kv[0][:], wkv_t[:, d, :],
                                     xfull[d][:, s0:s0 + SQC], start=st_, stop=sp)
                    mm_noload(kv[1][:], wkv_t[:, d, :],
                              xfull[d][:, s1:s1 + SQC], st_, sp)
                rope_kv(c0, kv[0], kv[1], 1)
                for h in range(2):
                    qt = [ps.tile([128, SQC], F32, name=f"qt{h}{j}", tag="proj")
                          for j in range(2)]
                    for d in range(NDC):
                        st_, sp = (d == 0), (d == NDC - 1)
                        w = wq_t[:, d, 128 * h:128 * h + 128]
                        nc.tensor.matmul(qt[0][:], w, xfull[d][:, s0:s0 + SQC],
                                         start=st_, stop=sp)
                        mm_noload(qt[1][:], w, xfull[d][:, s1:s1 + SQC],
                                  st_, sp)
                    rope_q(h, c0, qt)

            def emit_B(hp, j):
                """Attention for head pair hp, sq chunk j."""
                s0 = j * SQC
                q = qrot[hp]
                av = [ps.tile([DH + 1, SQC], F32, name=f"av{h}", tag="av")
                      for h in range(2)]
                nsk_j = 4 * j + 4
                for i in range(nsk_j):
                    k0 = i * SKC
                    m = i - 4 * j
                    off = 0 if m < 1 else 128 * m
                    nw = SQC - off
                    st2 = ps.tile([128, 2, SQC], F32, name="st2", tag="st",
                                  bufs=2)
                    for h in range(2):
                        r0 = 64 * h
                        nc.tensor.matmul(st2[:, h, off:SQC],
                                         krot[r0:r0 + 64, k0:k0 + SKC],
                                         q[r0:r0 + 64, s0 + off:s0 + SQC],
                                         start=True, stop=True,
                                         tile_position=(r0, 0))
                    pt = wpool.tile([128, 2, SQC], BF16, name="pt", tag="pt",
                                    bufs=6)
                    nc.scalar.activation(pt[:, :, off:SQC], st2[:, :, off:SQC],
                                         EXP, scale=0.125)
                    if m >= 0:
                        # causal triangle is only SKC wide: cols beyond
                        # off+SKC of a diagonal block are fully unmasked
                        nc.vector.tensor_mul(pt[:, :, off:off + SKC],
                                             pt[:, :, off:off + SKC],
                                             mask_t[:])
                    nc.tensor.matmul(av[0][:, off:SQC], vaug[:, i, :],
                                     pt[:, 0, off:SQC],
                                     start=(i == 0), stop=(i == nsk_j - 1))
                    mm_noload(av[1][:, off:SQC], vaug[:, i, :],
                              pt[:, 1, off:SQC],
                              (i == 0), (i == nsk_j - 1))

                # evacuate AV to SBUF right away (frees the av banks; the
                # softmax normalize then runs off the critical PSUM path)
                avf = wpool.tile([DH + 1, 2, SQC], F32, name="avf", tag="avf")
                for h in range(2):
                    nc.vector.tensor_copy(avf[:, h, :], av[h][:])

                # softmax normalize: 1/Z broadcast, write attnT
                for h in range(2):
                    nc.vector.tensor_copy(zg[32 * h:32 * h + 1, :],
                                          avf[64:65, h, :])
                nc.vector.reciprocal_approx_fast(zr[0:33, :], zg[0:33, :])
                for h in range(2):
                    if h == 0:
                        zsrc = zr[0:1, :]
                    else:
                        nc.vector.tensor_copy(z0[:], zr[32:33, :])
                        zsrc = z0[:]
                    bc = wpool.tile([64, SQC], F32, name="bc", tag="bc")
                    nc.gpsimd.partition_broadcast(bc[:], zsrc)
                    nc.vector.tensor_mul(attnT[hp][64 * h:64 * h + 64,
                                                   s0:s0 + SQC],
                                         avf[0:64, h, :], bc[:])

            def emit_C(si):
                """Output projection for seq rows [128si, 128si+128)."""
                stage = wpool.tile([128, D], BF16, name="cstage", tag="cstage")
                for op in range(2):       # oi pairs (0,1) and (2,3)
                    cps = [ps.tile([128, SQC], F32, name=f"cps{k}", tag="proj")
                           for k in range(2)]
                    for t in range(2):
                        aT = attnT[t][:, si * 128:(si + 1) * 128]
                        nc.tensor.matmul(cps[0][:], aT,
                                         wo_t[:, t, (2 * op) * SQC:
                                              (2 * op + 1) * SQC],
                                         start=(t == 0), stop=(t == 1))
                        mm_noload(cps[1][:], aT,
                                  wo_t[:, t, (2 * op + 1) * SQC:
                                       (2 * op + 2) * SQC],
                                  (t == 0), (t == 1))
                    # the last chunks' evacs go on ScalarE: exp work is done
                    # by then and DVE is busy with the final normalizes
                    for k in range(2):
                        o0 = (2 * op + k) * SQC
                        if 8 <= si < 12:
                            nc.scalar.copy(stage[:, o0:o0 + SQC], cps[k][:])
                        else:
                            nc.vector.tensor_copy(stage[:, o0:o0 + SQC],
                                                  cps[k][:])
                if 8 <= si < 12:
                    # tail chunks: drain in halves so the last DMA is smaller
                    nc.gpsimd.dma_start(out[si * 128:(si + 1) * 128, 0:1024],
                                        stage[:, 0:1024])
                    nc.gpsimd.dma_start(out[si * 128:(si + 1) * 128, 1024:D],
                                        stage[:, 1024:D])
                else:
                    nc.gpsimd.dma_start(out[si * 128:(si + 1) * 128, :],
                                        stage[:])

            # ---------------- the interleaved pipeline ----------------
            emit_dmas_pre()
            emit_A0()
            emit_B(0, 0)
            emit_B(0, 1)
            emit_A1()
            emit_B(1, 0)
            emit_B(1, 1)
            for si in range(0, 4):
                emit_C(si)
            emit_B(0, 2)
            emit_B(0, 3)
            for si in range(4, 8):
                emit_C(si)
            emit_B(1, 3)
            for si in range(12, 16):
                emit_C(si)
            emit_B(1, 2)
            for si in range(8, 12):
                emit_C(si)

    if noload:
        removed = _dedup_ldweights(nc)
        print(f"kernel: deduped {removed} redundant LDWEIGHTS", file=sys.stderr)
    nc.compile()
    return nc


def prep_in_maps(x, freqs_cos, freqs_sin, wq, wk, wv, wo):
    """Host-side sharding / pre-transposition. Returns list of 8 in_maps."""
    import ml_dtypes
    mmd = ml_dtypes.bfloat16

    x = np.asarray(x, dtype=np.float32)
    freqs_cos = np.asarray(freqs_cos, dtype=np.float32)
    freqs_sin = np.asarray(freqs_sin, dtype=np.float32)
    wq = np.asarray(wq, dtype=np.float32)
    wk = np.asarray(wk, dtype=np.float32)
    wv = np.asarray(wv, dtype=np.float32)
    wo = np.asarray(wo, dtype=np.float32)

    xT = np.ascontiguousarray(x.reshape(S, D).T).astype(mmd)   # [D, S]

    # head-dim permutation: even lanes first, odd lanes second
    perm = np.concatenate([np.arange(0, DH, 2), np.arange(1, DH, 2)])
    wq_h = wq.reshape(NH, DH, D)[:, perm, :]               # [NH, DH, D]
    wk_h = wk.reshape(NKV, DH, D)[:, perm, :]              # [NKV, DH, D]
    wv_h = wv.reshape(NKV, DH, D)                          # not permuted

    # cos rows tiled x4; sin rows: [-sin; +sin] tiled x2 (signs baked in)
    cosT = np.ascontiguousarray(freqs_cos.T)               # [32, S]
    sinT = np.ascontiguousarray(freqs_sin.T)
    cos4 = np.ascontiguousarray(np.tile(cosT, (4, 1))).astype(mmd)
    sin4 = np.ascontiguousarray(
        np.tile(np.concatenate([-sinT, sinT], axis=0), (2, 1))).astype(mmd)

    # causal triangle (the only partially-masked SKC columns of a diagonal
    # block), duplicated for the 2 heads of a pair: mask2[p, h, f] = f >= p
    p_idx = np.arange(128)[:, None, None]
    f_idx = np.arange(SKC)[None, None, :]
    mask2 = np.broadcast_to((f_idx >= p_idx), (128, 2, SKC)).astype(mmd)
    mask2 = np.ascontiguousarray(mask2)

    in_maps = []
    for c in range(NCORES):
        wq_c = wq_h[HQ * c:HQ * (c + 1)].reshape(HQ * DH, D)   # [256, D]
        wqT_c = np.ascontiguousarray(wq_c.T).astype(mmd)       # [D, 256]
        wq_int = np.ascontiguousarray(
            wqT_c.reshape(NDC, 128, HQ * DH).transpose(1, 0, 2))
        wkv_c = np.concatenate([wk_h[c], wv_h[c]], axis=0)     # [128, D]
        wkvT_c = np.ascontiguousarray(wkv_c.T).astype(mmd)     # [D, 128]
        wkv_int = np.ascontiguousarray(
            wkvT_c.reshape(NDC, 128, 2 * DH).transpose(1, 0, 2))
        woT_c = np.ascontiguousarray(
            wo[:, HQ * DH * c:HQ * DH * (c + 1)].T).astype(mmd)  # [256, D]
        wo_int = np.ascontiguousarray(
            woT_c.reshape(2, 128, D).transpose(1, 0, 2))
        in_maps.append({
            "xT": xT, "wq_il": wq_int, "wkv_il": wkv_int, "wo_il": wo_int,
            "cos4": cos4, "sin4": sin4, "mask2": mask2,
        })
    return in_maps


def run(inputs, trace=False, trace_cores=None, tmpdir=None):
    """Compile (cached), run on 8 cores, gather. Returns (output, results)."""
    nc = build_program()
    in_maps = prep_in_maps(**inputs)
    res = run_bass_kernel_spmd(nc, in_maps, core_ids=list(range(NCORES)),
                               trace=trace, trace_cores=trace_cores,
                               tmpdir=tmpdir)
    acc = np.zeros((S, D), dtype=np.float32)
    for r in res.results:
        acc += r["out"].astype(np.float32)
    return acc.reshape(1, S, D), res


def kernel(**inputs):
    out, _ = run(inputs)
    return out
